# revision 4
# baseline (speedup 1.0000x reference)
"""CASTLE forward kernel for 8 Trainium2 NeuronCores.

Shards the num_inputs (branch) axis: core c owns branches [8c, 8c+8).
x and the shared Linear are replicated; each core owns its slice of
mask_w / mask_b / W_out / b_out and the [B, I_local, H] activations.

Per-core math (branch i local index il, global i = 8c + il):
  masked_i = mask_w[i] with feature row i scaled by EPS        [64, 512]
  h0T_i    = gelu(masked_i^T x^T + mask_b[i])                  [512, B] (h on partitions)
  xiT_i    = W_share^T h0T_i + b_share                         [512, B]
  lT_i     = W_out[i]^T xiT_i                                  [2, B]
  Out[:,i] = sigmoid((l1-l0) + (bo1-bo0))                      (softmax last class, O=2)
Outputs: W (masked column norms), Out, out_layer[:,0,:], masked, out_sm[:,0,:].

All matmuls run in float32r (single-pass PE mode, ~17-bit mantissa).
"""

import os
import sys

for _p in ("/opt/trn_rl_repo", "/root/.axon_site/_ro/trn_rl_repo"):
    if os.path.isdir(_p) and _p not in sys.path:
        sys.path.insert(0, _p)

import numpy as np

I, H, O, B = 64, 512, 2, 2048
N_CORES, IL = 8, 8          # cores, local branches per core
EPS = 1e-12
BT = 512                    # batch tile (fp32 moving-operand max)
NBT = B // BT
NHC = H // 128              # 128-row chunks of the hidden dim

_CACHE = {}


# ---------------------------------------------------------------- build

def _build(repeats=1):
    import concourse.tile as tile
    from concourse import bacc, mybir

    f32, f32r = mybir.dt.float32, mybir.dt.float32r
    AF = mybir.ActivationFunctionType

    nc = bacc.Bacc("TRN2", target_bir_lowering=False, debug=False,
                   num_devices=N_CORES)

    # -------- per-core DRAM I/O (SPMD: same shapes, per-core data)
    xT_d = nc.dram_tensor("xT", [I, B], f32, kind="ExternalInput")
    mwT_d = nc.dram_tensor("mwT", [I, IL, H], f32, kind="ExternalInput")
    dsc_d = nc.dram_tensor("dscale", [I, IL], f32, kind="ExternalInput")
    mbT_d = nc.dram_tensor("mbT", [128, NHC, IL], f32, kind="ExternalInput")
    WsT_d = nc.dram_tensor("WsT", [128, NHC, H], f32, kind="ExternalInput")
    bshT_d = nc.dram_tensor("bshT", [128, NHC], f32, kind="ExternalInput")
    WoT_d = nc.dram_tensor("WoT", [128, IL, NHC, O], f32, kind="ExternalInput")
    bod_d = nc.dram_tensor("bod", [1, IL], f32, kind="ExternalInput")
    bodn_d = nc.dram_tensor("bodn", [1, IL], f32, kind="ExternalInput")
    bo01_d = nc.dram_tensor("bo01", [O, IL], f32, kind="ExternalInput")

    masked_d = nc.dram_tensor("masked_o", [I, IL, H], f32, kind="ExternalOutput")
    WT_d = nc.dram_tensor("WT", [I, IL], f32, kind="ExternalOutput")
    OutT_d = nc.dram_tensor("OutT", [IL, B], f32, kind="ExternalOutput")
    OL0_d = nc.dram_tensor("OL0", [O, B], f32, kind="ExternalOutput")
    OSM0_d = nc.dram_tensor("OSM0", [O, B], f32, kind="ExternalOutput")

    with tile.TileContext(nc) as tc:
        with (
            tc.tile_pool(name="consts", bufs=1) as consts,
            tc.tile_pool(name="h0", bufs=2) as h0_pool,
            tc.tile_pool(name="xi", bufs=2) as xi_pool,
            tc.tile_pool(name="small", bufs=8) as small,
            tc.tile_pool(name="psA", bufs=3, space="PSUM") as psA,
            tc.tile_pool(name="psB", bufs=2, space="PSUM") as psB,
            tc.tile_pool(name="psC", bufs=2, space="PSUM") as psC,
            tc.tile_pool(name="psL", bufs=1, space="PSUM") as psL,
        ):
            # -------- load constants
            xT_s = consts.tile([I, B], f32)
            nc.sync.dma_start(xT_s[:], xT_d[:])
            xTr = consts.tile([I, B], f32r)
            nc.vector.tensor_copy(xTr[:], xT_s[:])

            mw_s = consts.tile([I, IL, H], f32)
            nc.sync.dma_start(mw_s[:], mwT_d[:])
            dsc_s = consts.tile([I, IL], f32)
            nc.sync.dma_start(dsc_s[:], dsc_d[:])
            mb_s = consts.tile([128, NHC, IL], f32)
            nc.sync.dma_start(mb_s[:], mbT_d[:])

            Ws_s = consts.tile([128, NHC, H], f32)
            nc.sync.dma_start(Ws_s[:], WsT_d[:])
            Wsr = consts.tile([128, NHC, H], f32r)
            nc.vector.tensor_copy(Wsr[:], Ws_s[:])
            bsh_s = consts.tile([128, NHC], f32)
            nc.sync.dma_start(bsh_s[:], bshT_d[:])

            Wo_s = consts.tile([128, IL, NHC, O], f32)
            nc.sync.dma_start(Wo_s[:], WoT_d[:])
            Wor = consts.tile([128, IL, NHC, O], f32r)
            nc.vector.tensor_copy(Wor[:], Wo_s[:])
            # logit-diff weights: W_out[:,1] - W_out[:,0] (softmax-of-2 trick)
            Wod = consts.tile([128, IL, NHC], f32r)
            nc.vector.tensor_sub(Wod[:], Wo_s[:, :, :, 1], Wo_s[:, :, :, 0])

            bod_s = consts.tile([1, IL], f32)
            nc.sync.dma_start(bod_s[:], bod_d[:])
            bodn_s = consts.tile([1, IL], f32)
            nc.sync.dma_start(bodn_s[:], bodn_d[:])
            bo01_s = consts.tile([O, IL], f32)
            nc.sync.dma_start(bo01_s[:], bo01_d[:])

            # -------- masked weights, masked output, W column norms
            msk_s = consts.tile([I, IL, H], f32)
            mwr = consts.tile([I, IL, H], f32r)
            wsq = consts.tile([I, IL], f32)
            sq_scr = consts.tile([I, H], f32)
            for il in range(IL):
                nc.vector.tensor_scalar_mul(
                    msk_s[:, il, :], mw_s[:, il, :], dsc_s[:, il:il + 1])
                nc.vector.tensor_copy(mwr[:, il, :], msk_s[:, il, :])
                nc.sync.dma_start(masked_d[:, il, :], msk_s[:, il, :])
                nc.scalar.activation(sq_scr[:], msk_s[:, il, :], AF.Square,
                                     accum_out=wsq[:, il:il + 1])
            wt_s = consts.tile([I, IL], f32)
            nc.scalar.activation(wt_s[:], wsq[:], AF.Sqrt)
            nc.sync.dma_start(WT_d[:], wt_s[:])

            # -------- main batched pipeline
            for _rep in range(repeats):
                for bt in range(NBT):
                    xcol = xTr[:, bt * BT:(bt + 1) * BT]
                    for il in range(IL):
                        # stage A: h0T = gelu(masked^T @ xT + mask_b)
                        h0_sb = h0_pool.tile([128, NHC, BT], f32r, tag="h0")
                        for hc in range(NHC):
                            ph = psA.tile([128, BT], f32, tag="psA")
                            nc.tensor.matmul(
                                ph[:], mwr[:, il, hc * 128:(hc + 1) * 128],
                                xcol, start=True, stop=True)
                            nc.scalar.activation(
                                h0_sb[:, hc, :], ph[:], AF.Gelu,
                                bias=mb_s[:, hc, il:il + 1])
                        # stage B: xiT = W_share^T @ h0T + b_share
                        xi_sb = xi_pool.tile([128, NHC, BT], f32r, tag="xi")
                        for mc in range(NHC):
                            pxi = psB.tile([128, BT], f32, tag="psB")
                            for kc in range(NHC):
                                nc.tensor.matmul(
                                    pxi[:],
                                    Wsr[:, kc, mc * 128:(mc + 1) * 128],
                                    h0_sb[:, kc, :],
                                    start=(kc == 0), stop=(kc == NHC - 1))
                            nc.vector.tensor_scalar_add(
                                xi_sb[:, mc, :], pxi[:], bsh_s[:, mc:mc + 1])
                        # stage C: dT = (Wo1-Wo0)^T @ xiT  [1, BT]
                        # (softmax over 2 classes == sigmoid of logit diff)
                        pd = psC.tile([1, BT], f32, tag="psC")
                        for kc in range(NHC):
                            nc.tensor.matmul(
                                pd[:], Wod[:, il, kc:kc + 1], xi_sb[:, kc, :],
                                start=(kc == 0), stop=(kc == NHC - 1))
                        o = small.tile([1, BT], f32, tag="o")
                        nc.scalar.activation(o[:], pd[:], AF.Sigmoid,
                                             bias=bod_s[0:1, il:il + 1])
                        nc.sync.dma_start(
                            OutT_d[il:il + 1, bt * BT:(bt + 1) * BT], o[:])
                        if il == 0:
                            # branch-0 extras: raw logits + both softmax rows
                            pl = psL.tile([O, BT], f32, tag="psL")
                            for kc in range(NHC):
                                nc.tensor.matmul(
                                    pl[:], Wor[:, il, kc, :], xi_sb[:, kc, :],
                                    start=(kc == 0), stop=(kc == NHC - 1))
                            ol = small.tile([O, BT], f32, tag="ol")
                            nc.vector.tensor_scalar_add(
                                ol[:], pl[:], bo01_s[:, 0:1])
                            nc.sync.dma_start(
                                OL0_d[:, bt * BT:(bt + 1) * BT], ol[:])
                            om0 = small.tile([1, BT], f32, tag="om0")
                            nc.scalar.activation(om0[:], pd[:], AF.Sigmoid,
                                                 scale=-1.0,
                                                 bias=bodn_s[0:1, 0:1])
                            nc.sync.dma_start(
                                OSM0_d[0:1, bt * BT:(bt + 1) * BT], om0[:])
                            nc.sync.dma_start(
                                OSM0_d[1:2, bt * BT:(bt + 1) * BT], o[:])

    nc.compile()
    return nc


# ---------------------------------------------------------------- run

def _make_runner(nc):
    """jit-once runner: takes list of per-core input dicts, returns list of
    per-core output dicts. Modeled on bass2jax.run_bass_via_pjrt."""
    import jax
    from jax.sharding import Mesh, PartitionSpec
    from jax.experimental.shard_map import shard_map
    import concourse.mybir as mybir
    from concourse.bass2jax import (_bass_exec_p, install_neuronx_cc_hook,
                                    partition_id_tensor)

    install_neuronx_cc_hook()

    part_name = nc.partition_id_tensor.name if nc.partition_id_tensor else None
    in_names, out_names, out_avals = [], [], []
    for alloc in nc.m.functions[0].allocations:
        if not isinstance(alloc, mybir.MemoryLocationSet):
            continue
        name = alloc.memorylocations[0].name
        if alloc.kind == "ExternalInput":
            if name != part_name:
                in_names.append(name)
        elif alloc.kind == "ExternalOutput":
            out_names.append(name)
            out_avals.append(jax.core.ShapedArray(
                tuple(alloc.tensor_shape), mybir.dt.np(alloc.dtype)))
    n_params = len(in_names)
    all_names = in_names + out_names + ([part_name] if part_name else [])

    def _body(*args):
        operands = list(args)
        if part_name is not None:
            operands.append(partition_id_tensor())
        outs = _bass_exec_p.bind(
            *operands, out_avals=tuple(out_avals), in_names=tuple(all_names),
            out_names=tuple(out_names), lowering_input_output_aliases=(),
            sim_require_finite=True, sim_require_nnan=True, nc=nc)
        return tuple(outs)

    devices = jax.devices()[:N_CORES]
    mesh = Mesh(np.asarray(devices), ("core",))
    n_outs = len(out_names)
    sharded = jax.jit(shard_map(
        _body, mesh=mesh,
        in_specs=(PartitionSpec("core"),) * (n_params + n_outs),
        out_specs=(PartitionSpec("core"),) * n_outs, check_rep=False))

    zero_shapes = [tuple(a.shape) for a in out_avals]

    def run(in_maps):
        concat_in = [np.concatenate([np.asarray(in_maps[c][n])
                                     for c in range(N_CORES)], axis=0)
                     for n in in_names]
        concat_zeros = [np.zeros((N_CORES * s[0],) + s[1:], np.float32)
                        for s in zero_shapes]
        outs = sharded(*concat_in, *concat_zeros)
        return [
            {name: np.asarray(outs[i]).reshape((N_CORES,) + zero_shapes[i])[c]
             for i, name in enumerate(out_names)}
            for c in range(N_CORES)
        ]

    run.in_names = in_names
    run.out_names = out_names
    run.sharded = sharded
    run.n_params = n_params
    run.zero_shapes = zero_shapes
    return run


def _get_runner(repeats=1):
    key = ("runner", repeats)
    if key not in _CACHE:
        _CACHE[key] = _make_runner(_build(repeats))
    return _CACHE[key]


# ---------------------------------------------------------------- host glue

def _prep_inputs(x, mask_w, mask_b, W_share, b_share, W_out, b_out):
    f = np.float32
    x = np.asarray(x, f)
    mask_w = np.asarray(mask_w, f)
    mask_b = np.asarray(mask_b, f)
    W_share = np.asarray(W_share, f)
    b_share = np.asarray(b_share, f)
    W_out = np.asarray(W_out, f)
    b_out = np.asarray(b_out, f)

    xT = np.ascontiguousarray(x.T)                                   # [64, B]
    WsT = np.ascontiguousarray(
        W_share.reshape(NHC, 128, H).transpose(1, 0, 2))             # [128, 4, 512]
    bshT = np.ascontiguousarray(b_share.reshape(NHC, 128).T)         # [128, 4]

    in_maps = []
    for c in range(N_CORES):
        sl = slice(c * IL, (c + 1) * IL)
        mwT = np.ascontiguousarray(mask_w[sl].transpose(1, 0, 2))    # [64, 8, 512]
        dsc = np.ones((I, IL), f)
        for il in range(IL):
            dsc[c * IL + il, il] = EPS
        mbT = np.ascontiguousarray(
            mask_b[sl].reshape(IL, NHC, 128).transpose(2, 1, 0))     # [128, 4, 8]
        WoT = np.ascontiguousarray(
            W_out[sl].reshape(IL, NHC, 128, O).transpose(2, 0, 1, 3))  # [128,8,4,2]
        bod = (b_out[sl, 1] - b_out[sl, 0]).reshape(1, IL).astype(f)
        in_maps.append({
            "xT": xT, "mwT": mwT, "dscale": dsc, "mbT": mbT,
            "WsT": WsT, "bshT": bshT, "WoT": WoT,
            "bod": bod, "bodn": np.ascontiguousarray(-bod),
            "bo01": np.ascontiguousarray(b_out[sl].T),               # [2, 8]
        })
    return in_maps


def _assemble(results):
    masked = np.empty((I, I, H), np.float32)
    for c in range(N_CORES):
        masked[c * IL:(c + 1) * IL] = results[c]["masked_o"].transpose(1, 0, 2)
    W = np.concatenate([results[c]["WT"] for c in range(N_CORES)], axis=1)
    Out = np.concatenate([results[c]["OutT"] for c in range(N_CORES)],
                         axis=0).T.copy()
    ol0 = np.ascontiguousarray(results[0]["OL0"].T)
    osm0 = np.ascontiguousarray(results[0]["OSM0"].T)
    return W, Out, ol0, masked, osm0


def kernel(x, mask_w, mask_b, W_share, b_share, W_out, b_out):
    in_maps = _prep_inputs(x, mask_w, mask_b, W_share, b_share, W_out, b_out)
    run = _get_runner(repeats=1)
    results = run(in_maps)
    return _assemble(results)


# revision 11
# speedup vs baseline: 9.8254x; 9.8254x over previous
"""CASTLE forward kernel for 8 Trainium2 NeuronCores.

Shards the num_inputs (branch) axis: core c owns branches [8c, 8c+8).
x and the shared Linear are replicated; each core owns its slice of
mask_w / mask_b / W_out / b_out and the [B, I_local, H] activations.

Key algebra: x_i = h0 @ W_share + b_share is never an output — only its
O=2-dim projection through W_out is.  So the big [B,H]x[H,H] stage is
replaced by per-branch effective weights computed once:
    Weff_i = W_share @ W_out[i]            [512, 2]
    beff_i = b_share @ W_out[i] + b_out[i] [2]
    out_layer_i = h0_i @ Weff_i + beff_i
and softmax over 2 classes == sigmoid of the logit difference.

Per-core pipeline (branch i, local il):
  stage A: h0T = gelu([masked_i; mask_b_i]^T @ [x; 1]^T)  (bias as 65th row)
  stage C: dT_i = (Weff_i[:,1]-Weff_i[:,0])^T @ h0T       [1, B]
  end:     Out row il = sigmoid(dT_i + dbias_i), batched over branches

All matmuls in float32r (single-pass PE mode); all sigmoids are batched
after all gelus so ACT LUT-table reloads collapse to 3 (enforced with
explicit scheduler dep edges).
"""

import os
import sys

for _p in ("/opt/trn_rl_repo", "/root/.axon_site/_ro/trn_rl_repo"):
    if os.path.isdir(_p) and _p not in sys.path:
        sys.path.insert(0, _p)

import numpy as np

I, H, O, B = 64, 512, 2, 2048
N_CORES, IL = 8, 8          # cores, local branches per core
EPS = 1e-12
BT = 512                    # batch tile (fp32 moving-operand max)
NBT = B // BT
NHC = H // 128              # 128-row chunks of the hidden dim

_CACHE = {}


# ---------------------------------------------------------------- build

def _build(repeats=1):
    import concourse.tile as tile
    from concourse import bacc, mybir
    from concourse.bass import _add_dep_helper

    f32, f32r = mybir.dt.float32, mybir.dt.float32r
    AF = mybir.ActivationFunctionType

    nc = bacc.Bacc("TRN2", target_bir_lowering=False, debug=False,
                   num_devices=N_CORES)

    # -------- per-core DRAM I/O (SPMD: same shapes, per-core data)
    # x65 = [x^T; ones] so mask_b rides the contraction as a 65th row
    x65_d = nc.dram_tensor("x65", [I + 1, B], f32, kind="ExternalInput")
    mwT_d = nc.dram_tensor("mwT", [I, IL, H], f32, kind="ExternalInput")
    mb1_d = nc.dram_tensor("mb1", [1, IL, H], f32, kind="ExternalInput")
    dsc_d = nc.dram_tensor("dscale", [I, IL], f32, kind="ExternalInput")
    # W_share^T packed for the Weff matmuls: WsTT[p, kc, n] = Ws[n, kc*128+p]
    WsTT_d = nc.dram_tensor("WsTT", [128, NHC, H], f32, kind="ExternalInput")
    bshT_d = nc.dram_tensor("bshT", [128, NHC], f32, kind="ExternalInput")
    WoT_d = nc.dram_tensor("WoT", [128, IL, NHC, O], f32, kind="ExternalInput")
    bod8_d = nc.dram_tensor("bod8", [IL, 1], f32, kind="ExternalInput")
    bo01_d = nc.dram_tensor("bo01", [O, IL], f32, kind="ExternalInput")

    masked_d = nc.dram_tensor("masked_o", [I, IL, H], f32,
                              kind="ExternalOutput")
    WT_d = nc.dram_tensor("WT", [I, IL], f32, kind="ExternalOutput")
    OutT_d = nc.dram_tensor("OutT", [IL, B], f32, kind="ExternalOutput")
    OL0_d = nc.dram_tensor("OL0", [O, B], f32, kind="ExternalOutput")
    OSM0_d = nc.dram_tensor("OSM0", [O, B], f32, kind="ExternalOutput")

    with tile.TileContext(nc) as tc:
        with (
            tc.tile_pool(name="consts", bufs=1) as consts,
            tc.tile_pool(name="h0", bufs=3) as h0_pool,
            tc.tile_pool(name="dall", bufs=6) as dall_pool,
            tc.tile_pool(name="small", bufs=4) as small,
            tc.tile_pool(name="psA", bufs=2, space="PSUM") as psA,
            tc.tile_pool(name="psWF", bufs=1, space="PSUM") as psWF,
            tc.tile_pool(name="psC", bufs=2, space="PSUM") as psC,
            tc.tile_pool(name="psL", bufs=1, space="PSUM") as psL,
        ):
            # -------- load constants
            x65_s = consts.tile([I + 1, B], f32)
            nc.sync.dma_start(x65_s[:], x65_d[:])
            x65r = consts.tile([I + 1, B], f32r)
            nc.vector.tensor_copy(x65r[:], x65_s[:])

            # mw65[0:64] = mask_w slice, row 64 = mask_b (the bias row)
            mw65_s = consts.tile([I + 1, IL, H], f32)
            nc.sync.dma_start(mw65_s[0:I, :, :], mwT_d[:])
            nc.sync.dma_start(mw65_s[I:I + 1, :, :], mb1_d[:])
            dsc_s = consts.tile([I, IL], f32)
            nc.sync.dma_start(dsc_s[:], dsc_d[:])

            WsTT_s = consts.tile([128, NHC, H], f32)
            nc.sync.dma_start(WsTT_s[:], WsTT_d[:])
            WsTTr = consts.tile([128, NHC, H], f32r)
            nc.vector.tensor_copy(WsTTr[:], WsTT_s[:])
            bsh_s = consts.tile([128, NHC], f32)
            nc.sync.dma_start(bsh_s[:], bshT_d[:])
            # fp32r matmuls need N>=2: duplicate b_share into two rhs cols
            bshr2 = consts.tile([128, NHC, 2], f32r)
            bshc = bsh_s.rearrange("p (n o) -> p n o", o=1)
            nc.vector.tensor_copy(bshr2[:, :, 0:1], bshc)
            nc.vector.tensor_copy(bshr2[:, :, 1:2], bshc)

            Wo_s = consts.tile([128, IL, NHC, O], f32)
            nc.sync.dma_start(Wo_s[:], WoT_d[:])
            Wor = consts.tile([128, IL, NHC, O], f32r)
            nc.vector.tensor_copy(Wor[:], Wo_s[:])
            # logit-diff weights over the m dim (softmax-of-2 trick)
            Wod = consts.tile([128, IL, NHC], f32r)
            nc.vector.tensor_sub(Wod[:], Wo_s[:, :, :, 1], Wo_s[:, :, :, 0])

            bod8_s = consts.tile([IL, 1], f32)
            nc.sync.dma_start(bod8_s[:], bod8_d[:])
            bo01_s = consts.tile([O, IL], f32)
            nc.sync.dma_start(bo01_s[:], bo01_d[:])

            # -------- masked weights (scale diagonal rows), W column norms
            mw65r = consts.tile([I + 1, IL, H], f32r)
            wsq = consts.tile([I, IL], f32)
            sq_scr = consts.tile([I, H], f32)
            for il in range(IL):
                nc.vector.tensor_scalar_mul(
                    mw65_s[0:I, il, :], mw65_s[0:I, il, :],
                    dsc_s[:, il:il + 1])
                nc.sync.dma_start(masked_d[:, il, :], mw65_s[0:I, il, :])
                nc.scalar.activation(sq_scr[:], mw65_s[0:I, il, :], AF.Square,
                                     accum_out=wsq[:, il:il + 1])
            nc.vector.tensor_copy(mw65r[:], mw65_s[:])

            # -------- Weff = W_share @ W_out (all 8 branches: N=16 rhs)
            weff_r = consts.tile([128, NHC, IL * O], f32r)
            for hc in range(NHC):
                pw = psWF.tile([128, IL * O], f32, tag="wf")
                for kc in range(NHC):
                    nc.tensor.matmul(
                        pw[:], WsTTr[:, kc, hc * 128:(hc + 1) * 128],
                        Wor[:, :, kc, :], start=(kc == 0),
                        stop=(kc == NHC - 1))
                nc.vector.tensor_copy(weff_r[:, hc, :], pw[:])
            # per-branch logit-diff columns of Weff
            weff4 = weff_r.rearrange("p n (i o) -> p n i o", o=O)
            weffd = consts.tile([128, NHC, IL], f32r)
            nc.vector.tensor_sub(weffd[:], weff4[:, :, :, 1], weff4[:, :, :, 0])

            # -------- bias folds: dbias = b_share @ Wod + (bo1 - bo0);
            #          beff0 = b_share @ W_out[0] + b_out[0]
            pb8 = psWF.tile([IL, 2], f32, tag="wf")
            for kc in range(NHC):
                nc.tensor.matmul(pb8[:], Wod[:, :, kc], bshr2[:, kc, :],
                                 start=(kc == 0), stop=(kc == NHC - 1))
            dbias = consts.tile([IL, 1], f32)
            nc.vector.tensor_add(dbias[:], pb8[:, 0:1], bod8_s[:])

            pb0 = psWF.tile([O, 2], f32, tag="wf")
            for kc in range(NHC):
                nc.tensor.matmul(pb0[:], Wor[:, 0, kc, :], bshr2[:, kc, :],
                                 start=(kc == 0), stop=(kc == NHC - 1))
            beff0 = consts.tile([O, 1], f32)
            nc.vector.tensor_add(beff0[:], pb0[:, 0:1], bo01_s[:, 0:1])

            # -------- main batched pipeline (gelu table on ACT throughout)
            dstack = consts.tile([IL, B], f32)
            last_gelu = None
            for rep in range(repeats):
                for bt in range(NBT):
                    bsl = slice(bt * BT, (bt + 1) * BT)
                    for il in range(IL):
                        # stage A: h0T chunks, two PSUM banks per ACT op
                        h0_sb = h0_pool.tile([128, NHC, BT], f32r, tag="h0")
                        for hp in range(NHC // 2):
                            pa = psA.tile([128, 2, BT], f32, tag="psA")
                            for h2 in range(2):
                                hc = 2 * hp + h2
                                nc.tensor.matmul(
                                    pa[:, h2, :],
                                    mw65r[:, il, hc * 128:(hc + 1) * 128],
                                    x65r[:, bsl], start=True, stop=True)
                            g_i = nc.scalar.activation(
                                h0_sb[:, 2 * hp:2 * hp + 2, :], pa[:],
                                AF.Gelu)
                            last_gelu = g_i
                        # stage C: logit diff directly from h0
                        pd = psC.tile([1, BT], f32, tag="psC")
                        for kc in range(NHC):
                            nc.tensor.matmul(
                                pd[:], weffd[:, kc, il:il + 1],
                                h0_sb[:, kc, :],
                                start=(kc == 0), stop=(kc == NHC - 1))
                        dal = dall_pool.tile([1, BT], f32, tag="dall")
                        nc.vector.tensor_copy(dal[:], pd[:])
                        # scatter to the branch's partition for batched sigmoid
                        nc.sync.dma_start(dstack[il:il + 1, bsl], dal[:])
                        if il == 0:
                            # branch-0 raw logits (out_layer[:, 0, :])
                            pl = psL.tile([O, BT], f32, tag="psL")
                            for kc in range(NHC):
                                nc.tensor.matmul(
                                    pl[:], weff4[:, kc, 0, :],
                                    h0_sb[:, kc, :],
                                    start=(kc == 0), stop=(kc == NHC - 1))
                            ol = small.tile([O, BT], f32, tag="ol")
                            nc.vector.tensor_scalar_add(
                                ol[:], pl[:], beff0[:, 0:1])
                            nc.sync.dma_start(OL0_d[:, bsl], ol[:])

            # -------- end phase: batched sigmoids (one table swap), sqrt
            def after_gelus(inst):
                _add_dep_helper(inst.ins, last_gelu.ins, sync=True,
                                reason="batch sigmoids after gelus")

            os_t = consts.tile([IL, B], f32)
            s_i = nc.scalar.activation(os_t[:], dstack[:], AF.Sigmoid,
                                       bias=dbias[:])
            after_gelus(s_i)
            nc.sync.dma_start(OutT_d[:], os_t[:])
            nc.sync.dma_start(OSM0_d[1:2, :], os_t[0:1, :])
            om0 = consts.tile([1, B], f32)
            nc.vector.tensor_scalar(
                out=om0[:], in0=os_t[0:1, :], scalar1=-1.0, scalar2=1.0,
                op0=mybir.AluOpType.mult, op1=mybir.AluOpType.add)
            nc.sync.dma_start(OSM0_d[0:1, :], om0[:])

            wt_s = consts.tile([I, IL], f32)
            sq_i = nc.scalar.activation(wt_s[:], wsq[:], AF.Sqrt)
            after_gelus(sq_i)
            nc.sync.dma_start(WT_d[:], wt_s[:])

    nc.compile()
    return nc


# ---------------------------------------------------------------- run

def _make_runner(nc):
    """jit-once runner: takes list of per-core input dicts, returns list of
    per-core output dicts. Modeled on bass2jax.run_bass_via_pjrt."""
    import jax
    from jax.sharding import Mesh, PartitionSpec
    from jax.experimental.shard_map import shard_map
    import concourse.mybir as mybir
    from concourse.bass2jax import (_bass_exec_p, install_neuronx_cc_hook,
                                    partition_id_tensor)

    install_neuronx_cc_hook()

    part_name = nc.partition_id_tensor.name if nc.partition_id_tensor else None
    in_names, out_names, out_avals = [], [], []
    for alloc in nc.m.functions[0].allocations:
        if not isinstance(alloc, mybir.MemoryLocationSet):
            continue
        name = alloc.memorylocations[0].name
        if alloc.kind == "ExternalInput":
            if name != part_name:
                in_names.append(name)
        elif alloc.kind == "ExternalOutput":
            out_names.append(name)
            out_avals.append(jax.core.ShapedArray(
                tuple(alloc.tensor_shape), mybir.dt.np(alloc.dtype)))
    n_params = len(in_names)
    all_names = in_names + out_names + ([part_name] if part_name else [])

    def _body(*args):
        operands = list(args)
        if part_name is not None:
            operands.append(partition_id_tensor())
        outs = _bass_exec_p.bind(
            *operands, out_avals=tuple(out_avals), in_names=tuple(all_names),
            out_names=tuple(out_names), lowering_input_output_aliases=(),
            sim_require_finite=True, sim_require_nnan=True, nc=nc)
        return tuple(outs)

    devices = jax.devices()[:N_CORES]
    mesh = Mesh(np.asarray(devices), ("core",))
    n_outs = len(out_names)
    sharded = jax.jit(shard_map(
        _body, mesh=mesh,
        in_specs=(PartitionSpec("core"),) * (n_params + n_outs),
        out_specs=(PartitionSpec("core"),) * n_outs, check_rep=False))

    zero_shapes = [tuple(a.shape) for a in out_avals]

    def run(in_maps):
        concat_in = [np.concatenate([np.asarray(in_maps[c][n])
                                     for c in range(N_CORES)], axis=0)
                     for n in in_names]
        concat_zeros = [np.zeros((N_CORES * s[0],) + s[1:], np.float32)
                        for s in zero_shapes]
        outs = sharded(*concat_in, *concat_zeros)
        return [
            {name: np.asarray(outs[i]).reshape((N_CORES,) + zero_shapes[i])[c]
             for i, name in enumerate(out_names)}
            for c in range(N_CORES)
        ]

    run.in_names = in_names
    run.out_names = out_names
    run.sharded = sharded
    run.n_params = n_params
    run.zero_shapes = zero_shapes
    return run


def _get_runner(repeats=1):
    key = ("runner", repeats)
    if key not in _CACHE:
        _CACHE[key] = _make_runner(_build(repeats))
    return _CACHE[key]


# ---------------------------------------------------------------- host glue

def _prep_inputs(x, mask_w, mask_b, W_share, b_share, W_out, b_out):
    f = np.float32
    x = np.asarray(x, f)
    mask_w = np.asarray(mask_w, f)
    mask_b = np.asarray(mask_b, f)
    W_share = np.asarray(W_share, f)
    b_share = np.asarray(b_share, f)
    W_out = np.asarray(W_out, f)
    b_out = np.asarray(b_out, f)

    x65 = np.concatenate([x.T, np.ones((1, B), f)], axis=0)          # [65, B]
    WsTT = np.ascontiguousarray(
        W_share.T.reshape(NHC, 128, H).transpose(1, 0, 2))           # [128, 4, 512]
    bshT = np.ascontiguousarray(b_share.reshape(NHC, 128).T)         # [128, 4]

    in_maps = []
    for c in range(N_CORES):
        sl = slice(c * IL, (c + 1) * IL)
        mwT = np.ascontiguousarray(mask_w[sl].transpose(1, 0, 2))    # [64, 8, 512]
        mb1 = np.ascontiguousarray(mask_b[sl].reshape(1, IL, H))     # [1, 8, 512]
        dsc = np.ones((I, IL), f)
        for il in range(IL):
            dsc[c * IL + il, il] = EPS
        WoT = np.ascontiguousarray(
            W_out[sl].reshape(IL, NHC, 128, O).transpose(2, 0, 1, 3))  # [128,8,4,2]
        bod8 = (b_out[sl, 1] - b_out[sl, 0]).reshape(IL, 1).astype(f)
        in_maps.append({
            "x65": x65, "mwT": mwT, "mb1": mb1, "dscale": dsc,
            "WsTT": WsTT, "bshT": bshT, "WoT": WoT,
            "bod8": bod8,
            "bo01": np.ascontiguousarray(b_out[sl].T),               # [2, 8]
        })
    return in_maps


def _assemble(results):
    masked = np.empty((I, I, H), np.float32)
    for c in range(N_CORES):
        masked[c * IL:(c + 1) * IL] = results[c]["masked_o"].transpose(1, 0, 2)
    W = np.concatenate([results[c]["WT"] for c in range(N_CORES)], axis=1)
    Out = np.concatenate([results[c]["OutT"] for c in range(N_CORES)],
                         axis=0).T.copy()
    ol0 = np.ascontiguousarray(results[0]["OL0"].T)
    osm0 = np.ascontiguousarray(results[0]["OSM0"].T)
    return W, Out, ol0, masked, osm0


def kernel(x, mask_w, mask_b, W_share, b_share, W_out, b_out):
    in_maps = _prep_inputs(x, mask_w, mask_b, W_share, b_share, W_out, b_out)
    run = _get_runner(repeats=1)
    results = run(in_maps)
    return _assemble(results)
